# revision 2
# baseline (speedup 1.0000x reference)
"""Trainium2 Bass kernel for nn_BottomUpNet (dense_mlp).

Reference computation (per row n of N=8192, fully independent across rows):
    summary = aggregate (zeros, broadcast)            # (1024,)
    for k in 0..15:
        x = [summary, towers[n, k, :]]                # (1088,)
        h = relu(x @ OW1 + Ob1); h = relu(h @ OW2 + Ob2)
        pred_k = sigmoid(h @ OW3 + Ob3)
        m = relu(x @ MW1 + Mb1); m = relu(m @ MW2 + Mb2); m = relu(m @ MW3 + Mb3)
        summary = m
    out[n] = prod_k pred_k

Strategy: data-parallel over N across 8 cores (1024 rows each), weights
replicated.  Activations are kept feature-major ("transposed", [feature
partition, row free-dim]) so the f32 weight matrices serve directly as the
stationary matmul operand and no on-chip transposes are ever needed.
Matmuls run in bf16 (f32 PSUM accumulation; end-to-end rel err vs the f32
reference is ~7e-4, dominated by the bf16 rounding of the inputs), bias+relu
epilogues run on the scalar engine out of PSUM, and the K-step sigmoid
product is accumulated in f32 on the vector engine.
"""

import numpy as np
import ml_dtypes

import concourse.bacc as bacc
import concourse.mybir as mybir
import concourse.tile as tile
from concourse.bass import ts, ds
from concourse.bass_utils import run_bass_kernel_spmd

BF16 = ml_dtypes.bfloat16

N_CORES = 8
N = 8192
K = 16
NI = 64          # tower features per step
NH = 1024        # hidden width
FT = NH // 128   # feature tiles (8)
R = N // N_CORES  # rows per core (1024)
RB = 512         # row block (matmul moving dim / one PSUM bank)
NR = R // RB     # row blocks per core (2)

_BUILT = None


def _build():
    nc = bacc.Bacc("TRN2", target_bir_lowering=False, debug=False,
                   num_devices=N_CORES)
    f32 = mybir.dt.float32
    bf = mybir.dt.bfloat16

    towd = nc.declare_dram_parameter("tow", [K, NI, R], bf, isOutput=False)
    s0d = nc.declare_dram_parameter("s0", [NH, R], bf, isOutput=False)
    mw1sd = nc.declare_dram_parameter("mw1s", [NH, NH], bf, isOutput=False)
    mw1td = nc.declare_dram_parameter("mw1t", [NI, NH], bf, isOutput=False)
    mw2d = nc.declare_dram_parameter("mw2", [NH, NH], bf, isOutput=False)
    mw3d = nc.declare_dram_parameter("mw3", [NH, NH], bf, isOutput=False)
    ow1sd = nc.declare_dram_parameter("ow1s", [NH, NH], bf, isOutput=False)
    ow1td = nc.declare_dram_parameter("ow1t", [NI, NH], bf, isOutput=False)
    ow2d = nc.declare_dram_parameter("ow2", [NH, NH], bf, isOutput=False)
    ow3d = nc.declare_dram_parameter("ow3", [NH, 1], bf, isOutput=False)
    balld = nc.declare_dram_parameter("ball", [128, 40], f32, isOutput=False)
    ob3d = nc.declare_dram_parameter("ob3", [1, 1], f32, isOutput=False)
    outd = nc.declare_dram_parameter("out", [1, R], f32, isOutput=True)

    Relu = mybir.ActivationFunctionType.Relu
    Sigmoid = mybir.ActivationFunctionType.Sigmoid

    with tile.TileContext(nc) as tc:
        with (
            tc.tile_pool(name="weights", bufs=1) as wp,
            tc.tile_pool(name="summary", bufs=1) as sp,
            tc.tile_pool(name="acts", bufs=16) as ap,
            tc.tile_pool(name="tow", bufs=4) as twp,
            tc.tile_pool(name="small", bufs=1) as smp,
            tc.tile_pool(name="psum", bufs=6, space="PSUM") as pp,
            tc.tile_pool(name="zpsum", bufs=2, space="PSUM") as zp,
        ):
            # --- persistent weights ---
            def load_w(dram, name):
                tiles = []
                for i in range(FT):
                    t = wp.tile([128, NH], bf, tag=f"{name}{i}", name=f"{name}{i}")
                    nc.sync.dma_start(out=t, in_=dram[ts(i, 128), :])
                    tiles.append(t)
                return tiles

            mw1s = load_w(mw1sd, "mw1s")
            mw2 = load_w(mw2d, "mw2")
            mw3 = load_w(mw3d, "mw3")
            ow1s = load_w(ow1sd, "ow1s")
            ow2 = load_w(ow2d, "ow2")
            mw1t = wp.tile([NI, NH], bf, tag="mw1t", name="mw1t")
            nc.sync.dma_start(out=mw1t, in_=mw1td[:])
            ow1t = wp.tile([NI, NH], bf, tag="ow1t", name="ow1t")
            nc.sync.dma_start(out=ow1t, in_=ow1td[:])
            ow3 = []
            for i in range(FT):
                t = wp.tile([128, 1], bf, tag=f"ow3{i}", name=f"ow3{i}")
                nc.sync.dma_start(out=t, in_=ow3d[ts(i, 128), :])
                ow3.append(t)
            ball = smp.tile([128, 40], f32, tag="ball", name="ball")
            nc.sync.dma_start(out=ball, in_=balld[:])
            ob3 = smp.tile([1, 1], f32, tag="ob3", name="ob3")
            nc.sync.dma_start(out=ob3, in_=ob3d[:])

            # --- summary double buffer, initialized from s0 ---
            sA = [[sp.tile([128, RB], bf, tag=f"sA{i}_{r}", name=f"sA{i}_{r}") for r in range(NR)]
                  for i in range(FT)]
            sB = [[sp.tile([128, RB], bf, tag=f"sB{i}_{r}", name=f"sB{i}_{r}") for r in range(NR)]
                  for i in range(FT)]
            for i in range(FT):
                for r in range(NR):
                    nc.sync.dma_start(out=sA[i][r],
                                      in_=s0d[ts(i, 128), ts(r, RB)])

            # --- product accumulators ---
            pacc = []
            for r in range(NR):
                t = smp.tile([1, RB], f32, tag=f"pacc{r}", name=f"pacc{r}")
                nc.vector.memset(t, 1.0)
                pacc.append(t)

            # bias column index per layer: 0=Mb1 1=Mb2 2=Mb3 3=Ob1 4=Ob2
            def layer(rhs, ws, wt, tow_t, bias_l, out_tag, out_tiles=None,
                      out_dtype=bf):
                """One dense layer, feature-major: out[m][r] tiles."""
                outs = []
                for r in range(NR):
                    row = []
                    for m in range(FT):
                        ps = pp.tile([128, RB], mybir.dt.float32, tag="ps", name="ps")
                        nk = len(ws)
                        for i in range(nk):
                            nc.tensor.matmul(
                                ps[:], ws[i][:, ts(m, 128)], rhs[i][r][:],
                                start=(i == 0),
                                stop=(i == nk - 1 and wt is None),
                            )
                        if wt is not None:
                            nc.tensor.matmul(
                                ps[:], wt[:, ts(m, 128)],
                                tow_t[:, ts(r, RB)],
                                start=False, stop=True,
                            )
                        if out_tiles is not None:
                            ot = out_tiles[m][r]
                        else:
                            ot = ap.tile([128, RB], out_dtype, tag=out_tag, name=out_tag)
                        nc.scalar.activation(
                            ot[:], ps[:], Relu,
                            bias=ball[:, ds(bias_l * 8 + m, 1)])
                        row.append(ot)
                    outs.append(row)
                # outs[r][m] -> reindex to [i][r] for the next layer's rhs
                return [[outs[r][m] for r in range(NR)] for m in range(FT)]

            scur, snxt = sA, sB
            for k in range(K):
                tow_t = twp.tile([NI, R], bf, tag="tow", name="tow")
                nc.sync.dma_start(out=tow_t, in_=towd[k])

                # M branch (critical path)
                m1 = layer(scur, mw1s, mw1t, tow_t, 0, "l1")
                m2 = layer(m1, mw2, None, None, 1, "l2")
                layer(m2, mw3, None, None, 2, None, out_tiles=snxt)
                # O branch
                h1 = layer(scur, ow1s, ow1t, tow_t, 3, "l1")
                h2 = layer(h1, ow2, None, None, 4, "l2")
                # z = h2 @ OW3  -> sigmoid -> multiply into product
                for r in range(NR):
                    zps = zp.tile([1, RB], mybir.dt.float32, tag="z", name="z")
                    for i in range(FT):
                        nc.tensor.matmul(zps[:], ow3[i][:], h2[i][r][:],
                                         start=(i == 0), stop=(i == FT - 1))
                    pr = smp.tile([1, RB], mybir.dt.float32, tag=f"pr{r}", name=f"pr{r}")
                    nc.scalar.activation(pr[:], zps[:], Sigmoid, bias=ob3[:])
                    nc.vector.tensor_mul(pacc[r][:], pacc[r][:], pr[:])

                scur, snxt = snxt, scur

            for r in range(NR):
                nc.sync.dma_start(out=outd[:, ts(r, RB)], in_=pacc[r][:])

    nc.finalize()
    return nc


def _get_nc():
    global _BUILT
    if _BUILT is None:
        _BUILT = _build()
    return _BUILT


def _prep_inputs(inputs):
    f32 = np.float32
    towers = np.asarray(inputs["towers"], dtype=f32)
    agg = np.asarray(inputs["aggregate"], dtype=f32)
    MW1 = np.asarray(inputs["MW1"], dtype=f32)
    OW1 = np.asarray(inputs["OW1"], dtype=f32)

    shared = {
        "s0": np.ascontiguousarray(
            np.broadcast_to(agg.reshape(NH, 1), (NH, R))).astype(BF16),
        "mw1s": MW1[:NH].astype(BF16),
        "mw1t": np.ascontiguousarray(MW1[NH:]).astype(BF16),
        "mw2": np.asarray(inputs["MW2"], f32).astype(BF16),
        "mw3": np.asarray(inputs["MW3"], f32).astype(BF16),
        "ow1s": OW1[:NH].astype(BF16),
        "ow1t": np.ascontiguousarray(OW1[NH:]).astype(BF16),
        "ow2": np.asarray(inputs["OW2"], f32).astype(BF16),
        "ow3": np.asarray(inputs["OW3"], f32).astype(BF16),
        "ball": np.concatenate(
            [np.asarray(inputs[b], f32).reshape(FT, 128).T
             for b in ("Mb1", "Mb2", "Mb3", "Ob1", "Ob2")], axis=1),
        "ob3": np.asarray(inputs["Ob3"], f32).reshape(1, 1),
    }
    in_maps = []
    for c in range(N_CORES):
        tc_ = towers[c * R:(c + 1) * R]          # (R, K, NI)
        towT = np.ascontiguousarray(tc_.transpose(1, 2, 0)).astype(BF16)
        in_maps.append({"tow": towT, **shared})
    return in_maps


def _run(inputs, trace=False):
    nc = _get_nc()
    in_maps = _prep_inputs(inputs)
    res = run_bass_kernel_spmd(nc, in_maps, list(range(N_CORES)), trace=trace)
    out = np.concatenate([res.results[c]["out"][0] for c in range(N_CORES)])
    return out.astype(np.float32), res


def kernel(**inputs):
    out, _ = _run(inputs, trace=False)
    return out


# revision 5
# speedup vs baseline: 1.0667x; 1.0667x over previous
"""Trainium2 Bass kernel for nn_BottomUpNet (dense_mlp).

Reference computation (per row n of N=8192, fully independent across rows):
    summary = aggregate (broadcast)                   # (1024,)
    for k in 0..15:
        x = [summary, towers[n, k, :]]                # (1088,)
        h = relu(x @ OW1 + Ob1); h = relu(h @ OW2 + Ob2)
        pred_k = sigmoid(h @ OW3 + Ob3)
        m = relu(x @ MW1 + Mb1); m = relu(m @ MW2 + Mb2); m = relu(m @ MW3 + Mb3)
        summary = m
    out[n] = prod_k pred_k

Strategy: data-parallel over N across 8 cores (1024 rows each), weights
replicated.  Activations are feature-major ([feature partition, row free])
so weight matrices serve directly as the stationary matmul operand and no
on-chip transposes are needed.  Matmuls in bf16 with f32 PSUM accumulation
(end-to-end rel err vs the f32 reference ~8e-4); bias+relu epilogues on the
scalar engine out of PSUM.

Perf structure:
  - layer-1 tower matmuls (contraction 64) for the M- and O-branches are
    paired into disjoint PE row groups (0-63 / 64-127) so they run
    concurrently in the systolic array.
  - the 1024->1 output head is computed as a DVE per-partition
    multiply/add tree (g = sum_i h2_i * w3_i) followed by a single
    ones-vector matmul for the cross-partition reduce, instead of eight
    M=1 matmuls.
  - startup DMAs are spread across four DGE queues and ordered by first
    use; the initial summary broadcast is done on-chip from a 4KB vector.
"""

import numpy as np
import ml_dtypes

import concourse.bacc as bacc
import concourse.mybir as mybir
import concourse.tile as tile
from concourse.bass import ts, ds
from concourse.bass_utils import run_bass_kernel_spmd

BF16 = ml_dtypes.bfloat16

N_CORES = 8
N = 8192
K = 16
NI = 64          # tower features per step
NH = 1024        # hidden width
FT = NH // 128   # feature tiles (8)
R = N // N_CORES  # rows per core (1024)
RB = 512         # row block (matmul moving dim / one PSUM bank)
NR = R // RB     # row blocks per core (2)

_BUILT = None


def _build():
    nc = bacc.Bacc("TRN2", target_bir_lowering=False, debug=False,
                   num_devices=N_CORES)
    f32 = mybir.dt.float32
    bf = mybir.dt.bfloat16

    towd = nc.declare_dram_parameter("tow", [K, NI, R], bf, isOutput=False)
    aggd = nc.declare_dram_parameter("agg", [128, FT], f32, isOutput=False)
    mw1sd = nc.declare_dram_parameter("mw1s", [NH, NH], bf, isOutput=False)
    mw1td = nc.declare_dram_parameter("mw1t", [NI, NH], bf, isOutput=False)
    mw2d = nc.declare_dram_parameter("mw2", [NH, NH], bf, isOutput=False)
    mw3d = nc.declare_dram_parameter("mw3", [NH, NH], bf, isOutput=False)
    ow1sd = nc.declare_dram_parameter("ow1s", [NH, NH], bf, isOutput=False)
    ow1td = nc.declare_dram_parameter("ow1t", [NI, NH], bf, isOutput=False)
    ow2d = nc.declare_dram_parameter("ow2", [NH, NH], bf, isOutput=False)
    w3cd = nc.declare_dram_parameter("w3c", [128, FT], f32, isOutput=False)
    balld = nc.declare_dram_parameter("ball", [128, 40], f32, isOutput=False)
    ob3d = nc.declare_dram_parameter("ob3", [1, 1], f32, isOutput=False)
    outd = nc.declare_dram_parameter("out", [1, R], f32, isOutput=True)

    Relu = mybir.ActivationFunctionType.Relu
    Sigmoid = mybir.ActivationFunctionType.Sigmoid
    Add = mybir.AluOpType.add
    Mult = mybir.AluOpType.mult

    with tile.TileContext(nc) as tc:
        with (
            tc.tile_pool(name="weights", bufs=1) as wp,
            tc.tile_pool(name="summary", bufs=1) as sp,
            tc.tile_pool(name="acts", bufs=16) as ap,
            tc.tile_pool(name="tow", bufs=4) as twp,
            tc.tile_pool(name="small", bufs=1) as smp,
            tc.tile_pool(name="zwork", bufs=2) as zw,
            tc.tile_pool(name="psum", bufs=7, space="PSUM") as pp,
            tc.tile_pool(name="zpsum", bufs=1, space="PSUM") as zp,
        ):
            # --- weights, spread across DGE queues by first use ---
            def load_w(dram, name, eng):
                tiles = []
                for i in range(FT):
                    t = wp.tile([128, NH], bf, tag=f"{name}{i}",
                                name=f"{name}{i}")
                    eng.dma_start(out=t, in_=dram[ts(i, 128), :])
                    tiles.append(t)
                return tiles

            # sync (HW DGE): M-branch layer1 weights, needed first
            mw1s = load_w(mw1sd, "mw1s", nc.sync)
            mw1t = wp.tile([NI, NH], bf, tag="mw1t", name="mw1t")
            nc.sync.dma_start(out=mw1t, in_=mw1td[:])
            # scalar (HW DGE): small constants, then O layer1 weights,
            # then (emitted later) the per-step tower tiles
            ball = smp.tile([128, 40], f32, tag="ball", name="ball")
            nc.scalar.dma_start(out=ball, in_=balld[:])
            ob3 = smp.tile([1, 1], f32, tag="ob3", name="ob3")
            nc.scalar.dma_start(out=ob3, in_=ob3d[:])
            aggt = smp.tile([128, FT], f32, tag="aggt", name="aggt")
            nc.scalar.dma_start(out=aggt, in_=aggd[:])
            w3c = smp.tile([128, FT], f32, tag="w3c", name="w3c")
            nc.scalar.dma_start(out=w3c, in_=w3cd[:])
            ow1t = wp.tile([128, NH], bf, tag="ow1t", name="ow1t")
            nc.gpsimd.memset(ow1t, 0.0)
            nc.scalar.dma_start(out=ow1t[64:128, :], in_=ow1td[:])
            ow1s = load_w(ow1sd, "ow1s", nc.scalar)
            # gpsimd (SW DGE): layer 2/3 weights, needed tens of us in
            mw2 = load_w(mw2d, "mw2", nc.gpsimd)
            mw3 = load_w(mw3d, "mw3", nc.gpsimd)
            ow2 = load_w(ow2d, "ow2", nc.gpsimd)

            ones = smp.tile([128, 1], bf, tag="ones", name="ones")
            nc.vector.memset(ones, 1.0)
            zero = smp.tile([128, RB], bf, tag="zero", name="zero")
            nc.gpsimd.memset(zero, 0.0)

            # --- summary double buffer, init = broadcast of aggregate ---
            sA = [[sp.tile([128, RB], bf, tag=f"sA{i}_{r}",
                           name=f"sA{i}_{r}") for r in range(NR)]
                  for i in range(FT)]
            sB = [[sp.tile([128, RB], bf, tag=f"sB{i}_{r}",
                           name=f"sB{i}_{r}") for r in range(NR)]
                  for i in range(FT)]
            for r in range(NR):          # r0 first: layer1 r0 starts sooner
                for i in range(FT):
                    nc.vector.tensor_scalar(
                        sA[i][r][:], zero[:], aggt[:, ds(i, 1)], None, Add)

            # --- product accumulators ---
            pacc = []
            for r in range(NR):
                t = smp.tile([1, RB], f32, tag=f"pacc{r}", name=f"pacc{r}")
                nc.vector.memset(t, 1.0)
                pacc.append(t)

            # bias column index per layer: 0=Mb1 1=Mb2 2=Mb3 3=Ob1 4=Ob2
            def layer1(scur, tow_t):
                """Fused M/O layer 1.  Per (r, m): the M accumulation group
                ends with the contraction-64 tower matmul on PE rows 0-63,
                and the O group begins with its tower matmul on rows 64-127
                so the two stream concurrently in the array."""
                m1o, h1o = [], []
                for r in range(NR):
                    m1row, h1row = [], []
                    for m in range(FT):
                        psm = pp.tile([128, RB], mybir.dt.float32, tag="ps",
                                      name="psm")
                        pso = pp.tile([128, RB], mybir.dt.float32, tag="ps",
                                      name="pso")
                        for i in range(FT):
                            nc.tensor.matmul(
                                psm[:], mw1s[i][:, ts(m, 128)], scur[i][r][:],
                                start=(i == 0), stop=False)
                        nc.tensor.matmul(
                            psm[:], mw1t[:, ts(m, 128)],
                            tow_t[0:NI, ts(r, RB)], start=False, stop=True)
                        nc.tensor.matmul(
                            pso[:], ow1t[64:128, ts(m, 128)],
                            tow_t[64:128, ts(r, RB)], start=True, stop=False)
                        for i in range(FT):
                            nc.tensor.matmul(
                                pso[:], ow1s[i][:, ts(m, 128)], scur[i][r][:],
                                start=False, stop=(i == FT - 1))
                        m1t = ap.tile([128, RB], bf, tag="m1", name="m1")
                        nc.scalar.activation(m1t[:], psm[:], Relu,
                                             bias=ball[:, ds(m, 1)])
                        h1t = ap.tile([128, RB], bf, tag="h1", name="h1")
                        nc.scalar.activation(h1t[:], pso[:], Relu,
                                             bias=ball[:, ds(3 * 8 + m, 1)])
                        m1row.append(m1t)
                        h1row.append(h1t)
                    m1o.append(m1row)
                    h1o.append(h1row)
                reidx = lambda o: [[o[r][m] for r in range(NR)]
                                   for m in range(FT)]
                return reidx(m1o), reidx(h1o)

            def layer(rhs, ws, bias_l, out_tag, out_tiles=None):
                outs = []
                for r in range(NR):
                    row = []
                    for m in range(FT):
                        ps = pp.tile([128, RB], mybir.dt.float32, tag="ps",
                                     name="ps")
                        for i in range(FT):
                            nc.tensor.matmul(
                                ps[:], ws[i][:, ts(m, 128)], rhs[i][r][:],
                                start=(i == 0), stop=(i == FT - 1))
                        if out_tiles is not None:
                            ot = out_tiles[m][r]
                        else:
                            ot = ap.tile([128, RB], bf, tag=out_tag,
                                         name=out_tag)
                        nc.scalar.activation(ot[:], ps[:], Relu,
                                             bias=ball[:, ds(bias_l * 8 + m, 1)])
                        row.append(ot)
                    outs.append(row)
                return [[outs[r][m] for r in range(NR)] for m in range(FT)]

            scur, snxt = sA, sB
            for k in range(K):
                tow_t = twp.tile([128, R], bf, tag="tow", name="tow")
                nc.scalar.dma_start(out=tow_t[0:NI, :], in_=towd[k])
                nc.scalar.dma_start(out=tow_t[64:128, :], in_=towd[k])

                m1, h1 = layer1(scur, tow_t)
                m2 = layer(m1, mw2, 1, "l2")
                layer(m2, mw3, 2, None, out_tiles=snxt)
                h2 = layer(h1, ow2, 4, "l2")
                # output head: g = sum_i h2_i * w3_i (DVE), then a single
                # ones-matmul for the cross-partition reduce.
                for r in range(NR):
                    g = zw.tile([128, RB], mybir.dt.float32, tag="g",
                                name="g")
                    nc.vector.tensor_scalar(
                        g[:], h2[0][r][:], w3c[:, ds(0, 1)], None, Mult)
                    for i in range(1, FT):
                        t = zw.tile([128, RB], mybir.dt.float32, tag="t",
                                    name="t")
                        nc.vector.tensor_scalar(
                            t[:], h2[i][r][:], w3c[:, ds(i, 1)], None, Mult)
                        nc.vector.tensor_tensor(g[:], g[:], t[:], Add)
                    gb = zw.tile([128, RB], bf, tag="gb", name="gb")
                    nc.vector.tensor_copy(gb[:], g[:])
                    zps = zp.tile([1, RB], mybir.dt.float32, tag="z",
                                  name="zps")
                    nc.tensor.matmul(zps[:], ones[:], gb[:],
                                     start=True, stop=True)
                    pr = smp.tile([1, RB], mybir.dt.float32, tag=f"pr{r}",
                                  name=f"pr{r}")
                    nc.scalar.activation(pr[:], zps[:], Sigmoid, bias=ob3[:])
                    nc.vector.tensor_mul(pacc[r][:], pacc[r][:], pr[:])

                scur, snxt = snxt, scur

            for r in range(NR):
                nc.sync.dma_start(out=outd[:, ts(r, RB)], in_=pacc[r][:])

    nc.finalize()
    return nc


def _get_nc():
    global _BUILT
    if _BUILT is None:
        _BUILT = _build()
    return _BUILT


def _prep_inputs(inputs):
    f32 = np.float32
    towers = np.asarray(inputs["towers"], dtype=f32)
    agg = np.asarray(inputs["aggregate"], dtype=f32)
    MW1 = np.asarray(inputs["MW1"], dtype=f32)
    OW1 = np.asarray(inputs["OW1"], dtype=f32)

    shared = {
        "agg": np.ascontiguousarray(agg.reshape(FT, 128).T),
        "mw1s": MW1[:NH].astype(BF16),
        "mw1t": np.ascontiguousarray(MW1[NH:]).astype(BF16),
        "mw2": np.asarray(inputs["MW2"], f32).astype(BF16),
        "mw3": np.asarray(inputs["MW3"], f32).astype(BF16),
        "ow1s": OW1[:NH].astype(BF16),
        "ow1t": np.ascontiguousarray(OW1[NH:]).astype(BF16),
        "ow2": np.asarray(inputs["OW2"], f32).astype(BF16),
        "w3c": np.ascontiguousarray(
            np.asarray(inputs["OW3"], f32).reshape(FT, 128).T),
        "ball": np.concatenate(
            [np.asarray(inputs[b], f32).reshape(FT, 128).T
             for b in ("Mb1", "Mb2", "Mb3", "Ob1", "Ob2")], axis=1),
        "ob3": np.asarray(inputs["Ob3"], f32).reshape(1, 1),
    }
    in_maps = []
    for c in range(N_CORES):
        tc_ = towers[c * R:(c + 1) * R]          # (R, K, NI)
        towT = np.ascontiguousarray(tc_.transpose(1, 2, 0)).astype(BF16)
        in_maps.append({"tow": towT, **shared})
    return in_maps


def _run(inputs, trace=False):
    nc = _get_nc()
    in_maps = _prep_inputs(inputs)
    res = run_bass_kernel_spmd(nc, in_maps, list(range(N_CORES)), trace=trace)
    out = np.concatenate([res.results[c]["out"][0] for c in range(N_CORES)])
    return out.astype(np.float32), res


def kernel(**inputs):
    out, _ = _run(inputs, trace=False)
    return out
